# revision 9
# baseline (speedup 1.0000x reference)
"""Trainium2 Bass kernel for nn_Decoder_55164559950059 (sparse deformable-attention decoder).

Strategy (data-parallel over batch, one batch element per NeuronCore):
  - Per core: evaluate sampling polynomial on DVE, derive bilinear gather
    indices/weights on DVE, gather 2x2 pixel patches (as two 2-row pair
    fetches) straight from HBM with indirect DMA, combine corners with
    per-partition scalar multiplies, transpose emb via the PE, run the
    2-layer tanh MLP on the PE, and assemble the final reference points.
  - Row ordering: r = s*512 + q (q padded 500->512). Row r lives at
    partition p = q%128... precisely q = qb*128 + p, gather/compute column
    col = s*4 + qb, so tile [128, col] layouts hold everything.

kernel(**inputs) takes the FULL unsharded inputs and returns the full
(8, 500, 16, 2) float32 output.
"""

import sys
from contextlib import ExitStack

import numpy as np

sys.path.insert(0, "/opt/trn_rl_repo")

import concourse.bacc as bacc
import concourse.bass as bass
import concourse.mybir as mybir
import concourse.tile as tile
from concourse.bass_utils import run_bass_kernel_spmd
from concourse.masks import make_identity

from concourse import library_config

f32 = mybir.dt.float32
i32 = mybir.dt.int32
i16 = mybir.dt.int16
Alu = mybir.AluOpType
ActF = mybir.ActivationFunctionType

# ---- problem constants (hardcoded per the harness contract) ----
DEGREE = 3
S = 8                      # sampling points
NP = 2                     # deformable offsets per point
S_SCALE = 0.077
SPATIAL = [(80, 80), (40, 40), (20, 20), (10, 10)]
STARTS = [0, 6400, 8000, 8400]
B, Q, C = 8, 500, 256
HWR = 8500                 # total memory rows per batch
QP = 512                   # padded queries
NQB = QP // 128            # 4 query blocks
NCOL = S * NQB             # 32 chunk columns, col = s*NQB + qb
NGRP = NCOL // 4           # 8 groups of 4 columns (512 rows each)
TWO23 = 12582912.0         # 1.5*2**23 fp32 round-to-int bias (spacing 1.0 at the sum)

N_CORES = 8


def _lambdas_np():
    lam = np.linspace(0.0, 1.0, S, dtype=np.float32)
    return np.stack([lam ** i for i in range(DEGREE, -1, -1)], 1)  # (S, 4)


def _level_quantity(vals):
    """vals[l] for l in 0..3 -> list of (level, value) floats."""
    return [float(v) for v in vals]


def build_nc():
    nc = bacc.Bacc("TRN2", target_bir_lowering=False, debug=False)

    mem = nc.dram_tensor("mem", [HWR, C], f32, kind="ExternalInput")
    rp = nc.dram_tensor("rp", [Q, 2 * (DEGREE + 1)], f32, kind="ExternalInput")
    lv = nc.dram_tensor("lv", [Q], i32, kind="ExternalInput")
    w1 = nc.dram_tensor("w1", [C, C], f32, kind="ExternalInput")
    b1 = nc.dram_tensor("b1", [C], f32, kind="ExternalInput")
    w2 = nc.dram_tensor("w2", [C, 2 * NP], f32, kind="ExternalInput")
    b2r = nc.dram_tensor("b2r", [128, NCOL * 4], f32, kind="ExternalInput")
    lamr = nc.dram_tensor("lamr", [128, 256], f32, kind="ExternalInput")
    out = nc.dram_tensor("out", [Q, S, NP, 2], f32, kind="ExternalOutput")

    with tile.TileContext(nc) as tc:
        with ExitStack() as ctx:
            _body(ctx, tc, mem, rp, lv, w1, b1, w2, b2r, lamr, out)

    nc.compile()
    return nc


def _body(ctx, tc, mem, rp, lv, w1, b1, w2, b2r, lamr, out):
    nc = tc.nc
    v = nc.vector
    sc = nc.scalar
    gp = nc.gpsimd

    cpool = ctx.enter_context(tc.tile_pool(name="consts", bufs=1))
    spool = ctx.enter_context(tc.tile_pool(name="scratch", bufs=1))
    gpool = ctx.enter_context(tc.tile_pool(name="gather", bufs=4))
    epool = ctx.enter_context(tc.tile_pool(name="emb", bufs=3))
    pepool = ctx.enter_context(tc.tile_pool(name="pe_t", bufs=2, space="PSUM"))
    esb = ctx.enter_context(tc.tile_pool(name="embT", bufs=2))
    hpsum = ctx.enter_context(tc.tile_pool(name="hpsum", bufs=2, space="PSUM"))
    hsb = ctx.enter_context(tc.tile_pool(name="hT", bufs=2))
    opsum = ctx.enter_context(tc.tile_pool(name="opsum", bufs=1, space="PSUM"))
    fpool = ctx.enter_context(tc.tile_pool(name="final", bufs=1))

    # ---------------- constants & weights ----------------
    gp.load_library(library_config.mlp)

    ident = cpool.tile([128, 128], f32, tag="ident")
    make_identity(nc, ident[:, :])

    w1_sb = [cpool.tile([128, C], f32, tag=f"w1_{kh}", name=f"w1_sb{kh}") for kh in range(2)]
    for kh in range(2):
        nc.sync.dma_start(w1_sb[kh][:, :], w1[kh * 128:(kh + 1) * 128, :])
    w2_sb = [cpool.tile([128, 2 * NP], f32, tag=f"w2_{kh}", name=f"w2_sb{kh}") for kh in range(2)]
    for kh in range(2):
        nc.sync.dma_start(w2_sb[kh][:, :], w2[kh * 128:(kh + 1) * 128, :])
    b1_sb = cpool.tile([128, 2], f32, tag="b1")
    nc.sync.dma_start(b1_sb[:, :], b1.rearrange("(h p) -> p h", p=128))
    b2r_sb = cpool.tile([128, NCOL * 4], f32, tag="b2r")
    nc.sync.dma_start(b2r_sb[:, :], b2r[:, :])
    lam_sb = cpool.tile([128, 256], f32, tag="lamr")
    nc.sync.dma_start(lam_sb[:, :], lamr[:, :])

    # ---------------- small input loads (ragged 500 -> [128, 4]) ----------------
    rp_t = spool.tile([128, NQB, 8], f32, tag="rp_t")
    v.memset(rp_t[:, :, :], 0.0)
    nc.sync.dma_start(rp_t[:, 0:3, :], rp[0:384, :].rearrange("(a p) m -> p a m", p=128))
    nc.sync.dma_start(rp_t[0:116, 3, :], rp[384:500, :])

    lv_t = spool.tile([128, NQB], i32, tag="lv_t")
    v.memset(lv_t[:, :], 0)
    nc.sync.dma_start(lv_t[:, 0:3], lv[0:384].rearrange("(a p) -> p a", p=128))
    nc.sync.dma_start(lv_t[0:116, 3], lv[384:500])

    # ---------------- polynomial eval: e[p, s, qb, c] ----------------
    rp_rep = spool.tile([128, S, NQB, 2, DEGREE + 1], f32, tag="rp_rep")
    for s in range(S):
        v.tensor_copy(rp_rep[:, s, :, :, :], rp_t[:, :, :].rearrange("p a (c d) -> p a c d", c=2))
    prod = spool.tile([128, S, NQB, 2, DEGREE + 1], f32, tag="prod")
    v.tensor_tensor(
        out=prod[:, :, :, :, :],
        in0=rp_rep[:, :, :, :, :],
        in1=lam_sb[:, :].rearrange("p (s a c d) -> p s a c d", s=S, a=NQB, c=2),
        op=Alu.mult,
    )
    e_t = spool.tile([128, S, NQB, 2], f32, tag="e_t")
    v.tensor_reduce(e_t[:, :, :, :], prod[:, :, :, :, :], axis=mybir.AxisListType.X, op=Alu.add)

    # sp = 2*e - 1 (sampling points in [-1, 1], layout [p, s, qb, c])
    sp_t = spool.tile([128, S, NQB, 2], f32, tag="sp_t")
    v.tensor_scalar(out=sp_t[:, :, :, :], in0=e_t[:, :, :, :],
                    scalar1=2.0, scalar2=-1.0, op0=Alu.mult, op1=Alu.add)

    # ---------------- per-query level constants at [128, NCOL] ----------------
    lv_f = spool.tile([128, NQB], f32, tag="lv_f")
    v.tensor_copy(lv_f[:, :], lv_t[:, :])
    lv_rep = spool.tile([128, S, NQB], f32, tag="lv_rep")
    for s in range(S):
        v.tensor_copy(lv_rep[:, s, :], lv_f[:, :])

    def level_tile(tag, vals):
        acc = spool.tile([128, S, NQB], f32, tag=tag)
        tmp = spool.tile([128, S, NQB], f32, tag=tag + "_tmp")
        for l in range(4):
            dst = acc if l == 0 else tmp
            v.tensor_scalar(out=dst[:, :, :], in0=lv_rep[:, :, :],
                            scalar1=float(l), scalar2=float(vals[l]),
                            op0=Alu.is_equal, op1=Alu.mult)
            if l > 0:
                v.tensor_tensor(out=acc[:, :, :], in0=acc[:, :, :], in1=tmp[:, :, :], op=Alu.add)
        return acc

    Wq = level_tile("Wq", [wd for (_, wd) in SPATIAL])
    Hq = level_tile("Hq", [h for (h, _) in SPATIAL])
    Wm2 = level_tile("Wm2", [wd - 2 for (_, wd) in SPATIAL])
    Hm2 = level_tile("Hm2", [h - 2 for (h, _) in SPATIAL])
    STq = level_tile("STq", STARTS)

    # ---------------- bilinear index / weight math ----------------
    def axis_math(e_comp, SZ, SZm2, tag):
        """e_comp: [128, S, NQB] view of e (x or y). Returns (lo, wA, wB)."""
        g = spool.tile([128, S, NQB], f32, tag=tag + "_g")
        v.tensor_tensor(out=g[:, :, :], in0=e_comp, in1=SZ[:, :, :], op=Alu.mult)
        v.tensor_scalar_add(g[:, :, :], g[:, :, :], -0.5)
        # exact floor via 2^23 round + correction
        r = spool.tile([128, S, NQB], f32, tag=tag + "_r")
        v.tensor_scalar(out=r[:, :, :], in0=g[:, :, :], scalar1=TWO23, scalar2=TWO23,
                        op0=Alu.add, op1=Alu.subtract)
        cmp = spool.tile([128, S, NQB], f32, tag=tag + "_c")
        v.tensor_tensor(out=cmp[:, :, :], in0=r[:, :, :], in1=g[:, :, :], op=Alu.is_gt)
        x0 = spool.tile([128, S, NQB], f32, tag=tag + "_x0")
        v.tensor_tensor(out=x0[:, :, :], in0=r[:, :, :], in1=cmp[:, :, :], op=Alu.subtract)
        fx = spool.tile([128, S, NQB], f32, tag=tag + "_f")
        v.tensor_tensor(out=fx[:, :, :], in0=g[:, :, :], in1=x0[:, :, :], op=Alu.subtract)
        lo = spool.tile([128, S, NQB], f32, tag=tag + "_lo")
        v.tensor_scalar_max(lo[:, :, :], x0[:, :, :], 0.0)
        v.tensor_tensor(out=lo[:, :, :], in0=lo[:, :, :], in1=SZm2[:, :, :], op=Alu.min)
        x0p = spool.tile([128, S, NQB], f32, tag=tag + "_x0p")
        v.tensor_scalar_add(x0p[:, :, :], x0[:, :, :], 1.0)
        lop = spool.tile([128, S, NQB], f32, tag=tag + "_lop")
        v.tensor_scalar_add(lop[:, :, :], lo[:, :, :], 1.0)

        def eqw(a, b, tg):
            t = spool.tile([128, S, NQB], f32, tag=tg)
            v.tensor_tensor(out=t[:, :, :], in0=a[:, :, :], in1=b[:, :, :], op=Alu.is_equal)
            return t

        e1 = eqw(lo, x0, tag + "_e1")
        e2 = eqw(lo, x0p, tag + "_e2")
        e3 = eqw(lop, x0, tag + "_e3")
        e4 = eqw(lop, x0p, tag + "_e4")

        def blend(ea, eb, tg):
            # w = ea + fx*(eb - ea)
            d = spool.tile([128, S, NQB], f32, tag=tg + "_d")
            v.tensor_tensor(out=d[:, :, :], in0=eb[:, :, :], in1=ea[:, :, :], op=Alu.subtract)
            v.tensor_tensor(out=d[:, :, :], in0=fx[:, :, :], in1=d[:, :, :], op=Alu.mult)
            v.tensor_tensor(out=d[:, :, :], in0=ea[:, :, :], in1=d[:, :, :], op=Alu.add)
            return d

        wA = blend(e1, e2, tag + "_wA")
        wB = blend(e3, e4, tag + "_wB")
        return lo, wA, wB

    xlo, wxA, wxB = axis_math(e_t[:, :, :, 0], Wq, Wm2, "x")
    ylo, wyA, wyB = axis_math(e_t[:, :, :, 1], Hq, Hm2, "y")

    def outer(wy, wx, tg):
        t = spool.tile([128, S, NQB], f32, tag=tg)
        v.tensor_tensor(out=t[:, :, :], in0=wy[:, :, :], in1=wx[:, :, :], op=Alu.mult)
        return t

    wAA = outer(wyA, wxA, "wAA")   # row yl,   col xl / xl+1
    wAB = outer(wyA, wxB, "wAB")
    wBA = outer(wyB, wxA, "wBA")
    wBB = outer(wyB, wxB, "wBB")

    rowA = spool.tile([128, S, NQB], f32, tag="rowA")
    v.tensor_tensor(out=rowA[:, :, :], in0=ylo[:, :, :], in1=Wq[:, :, :], op=Alu.mult)
    v.tensor_tensor(out=rowA[:, :, :], in0=rowA[:, :, :], in1=STq[:, :, :], op=Alu.add)
    v.tensor_tensor(out=rowA[:, :, :], in0=rowA[:, :, :], in1=xlo[:, :, :], op=Alu.add)
    rowB = spool.tile([128, S, NQB], f32, tag="rowB")
    v.tensor_tensor(out=rowB[:, :, :], in0=rowA[:, :, :], in1=Wq[:, :, :], op=Alu.add)

    # int16 indices, free layout (s, qb, ab) -> global gather position
    # I = p + 128*(8*s + 2*qb + ab); dma_gather wants wrapped[I%16, I//16]
    # replicated across the 8 Q7 core groups.
    idx16 = spool.tile([128, S, NQB, 2], i16, tag="idx16")
    v.tensor_copy(idx16[:, :, :, 0], rowA[:, :, :])
    v.tensor_copy(idx16[:, :, :, 1], rowB[:, :, :])
    idx16_f = idx16[:, :, :, :].rearrange("p s a j -> p (s a j)")  # [128, 64]
    w16 = spool.tile([16, 64, 8], i16, tag="w16")  # [pl, cglobal, pg]
    for pg in range(8):
        nc.sync.dma_start(w16[:, :, pg], idx16_f[16 * pg:16 * (pg + 1), :])
    idxs_t = spool.tile([128, 512], i16, tag="idxs_t")
    w16_f = w16[:, :, :].rearrange("p c g -> p (c g)")  # [16, 512]
    for k in range(8):
        nc.sync.dma_start(idxs_t[16 * k:16 * (k + 1), :], w16_f)

    mem_pairs = bass.AP(tensor=mem[:, :].tensor, offset=0, ap=[[C, HWR - 1], [1, 2 * C]])

    wAA_f = wAA[:, :, :].rearrange("p s a -> p (s a)")
    wAB_f = wAB[:, :, :].rearrange("p s a -> p (s a)")
    wBA_f = wBA[:, :, :].rearrange("p s a -> p (s a)")
    wBB_f = wBB[:, :, :].rearrange("p s a -> p (s a)")

    # ---------------- main loop: gather -> combine -> transpose -> MLP ----------------
    psum_off = opsum.tile([128, NCOL * 4], f32, tag="psum_off")

    for g in range(NGRP):
        embT_sb = [esb.tile([128, 512], f32, tag=f"embT_{kh}", name=f"embT_sb{kh}_{g}") for kh in range(2)]
        g_t = gpool.tile([128, 8, 2 * C], f32, tag="g_t")
        gp.dma_gather(
            out_ap=g_t[:, :, :],
            in_ap=mem_pairs,
            idxs_ap=idxs_t[:, 64 * g:64 * (g + 1)],
            num_idxs=1024,
            num_idxs_reg=1024,
            elem_size=2 * C,
            elem_step=C,
        )
        for j in range(4):
            col = 4 * g + j
            # bilinear combine: emb = wAA*A0 + wAB*A1 + wBA*B0 + wBB*B1
            emb = epool.tile([128, C], f32, tag="emb")
            t1 = epool.tile([128, C], f32, tag="t1")
            t2 = epool.tile([128, C], f32, tag="t2")
            t3 = epool.tile([128, C], f32, tag="t3")
            v.tensor_scalar_mul(emb[:, :], g_t[:, 2 * j, 0:C], wAA_f[:, col:col + 1])
            sc.mul(t1[:, :], g_t[:, 2 * j, C:2 * C], wAB_f[:, col:col + 1])
            v.tensor_scalar_mul(t2[:, :], g_t[:, 2 * j + 1, 0:C], wBA_f[:, col:col + 1])
            sc.mul(t3[:, :], g_t[:, 2 * j + 1, C:2 * C], wBB_f[:, col:col + 1])
            v.tensor_tensor(out=emb[:, :], in0=emb[:, :], in1=t1[:, :], op=Alu.add)
            v.tensor_tensor(out=t2[:, :], in0=t2[:, :], in1=t3[:, :], op=Alu.add)
            v.tensor_tensor(out=emb[:, :], in0=emb[:, :], in1=t2[:, :], op=Alu.add)
            # transpose emb [rows, C] -> embT [C, rows] via PE
            pet = pepool.tile([128, 2, 128], f32, tag="pet")
            for kh in range(2):
                nc.tensor.transpose(pet[:, kh, :], emb[:, kh * 128:(kh + 1) * 128], ident[:, :])
            v.tensor_copy(embT_sb[0][:, j * 128:(j + 1) * 128], pet[:, 0, :])
            sc.copy(embT_sb[1][:, j * 128:(j + 1) * 128], pet[:, 1, :])

        # MLP layer 1: h = tanh(emb @ W1 + b1), computed transposed
        hT_sb = [hsb.tile([128, 512], f32, tag=f"hT_{mh}", name=f"hT_sb{mh}_{g}") for mh in range(2)]
        for mh in range(2):
            ph = hpsum.tile([128, 512], f32, tag=f"ph_{mh}")
            for kh in range(2):
                nc.tensor.matmul(
                    ph[:, :],
                    lhsT=w1_sb[kh][:, mh * 128:(mh + 1) * 128],
                    rhs=embT_sb[kh][:, :],
                    start=(kh == 0),
                    stop=(kh == 1),
                )
            sc.activation(hT_sb[mh][:, :], ph[:, :], ActF.Tanh, bias=b1_sb[:, mh:mh + 1], scale=1.0)

        # MLP layer 2 (swapped operands): off[rows, 4] directly
        for j in range(4):
            col = 4 * g + j
            for kh in range(2):
                nc.tensor.matmul(
                    psum_off[:, col * 4:(col + 1) * 4],
                    lhsT=hT_sb[kh][:, j * 128:(j + 1) * 128],
                    rhs=w2_sb[kh][:, :],
                    start=(kh == 0),
                    stop=(kh == 1),
                )

    # ---------------- final: ref = S_SCALE*tanh(off + b2) + sp ----------------
    spts4 = fpool.tile([128, NCOL, NP, 2], f32, tag="spts4")
    sp_cols = sp_t[:, :, :, :].rearrange("p s a c -> p (s a) c")
    for np_i in range(NP):
        v.tensor_copy(spts4[:, :, np_i, :], sp_cols)

    so1 = fpool.tile([128, NCOL * 4], f32, tag="so1")
    v.tensor_tensor(out=so1[:, :], in0=psum_off[:, :], in1=b2r_sb[:, :], op=Alu.add)
    so2 = fpool.tile([128, NCOL * 4], f32, tag="so2")
    sc.activation(so2[:, :], so1[:, :], ActF.Tanh)
    ref_out = fpool.tile([128, NCOL, NP, 2], f32, tag="ref_out")
    v.tensor_scalar_mul(ref_out[:, :, :, :],
                        so2[:, :].rearrange("p (k n c) -> p k n c", n=NP, c=2),
                        S_SCALE)
    v.tensor_tensor(out=ref_out[:, :, :, :], in0=ref_out[:, :, :, :],
                    in1=spts4[:, :, :, :], op=Alu.add)

    # ---------------- output DMA (ragged 512 -> 500) ----------------
    ro = ref_out[:, :, :, :].rearrange("p (s a) n c -> p s a (n c)", s=S)
    for qb in range(NQB):
        pmax = 128 if qb < 3 else 116
        nc.sync.dma_start(
            out[qb * 128:qb * 128 + pmax, :, :, :].rearrange("p s n c -> p s (n c)"),
            ro[0:pmax, :, qb, :],
        )


# ---------------- host wrapper ----------------
_NC_CACHE = {}


def _get_nc():
    if "nc" not in _NC_CACHE:
        _NC_CACHE["nc"] = build_nc()
    return _NC_CACHE["nc"]


def make_in_maps(ref_polys, ref_levels, memory, W1, b1, W2, b2):
    lam = _lambdas_np()  # (S, 4)
    lamr = np.broadcast_to(lam[None, :, None, None, :], (128, S, NQB, 2, DEGREE + 1))
    lamr = np.ascontiguousarray(lamr.reshape(128, 256).astype(np.float32))
    b2r = np.broadcast_to(np.asarray(b2, np.float32)[None, None, :], (128, NCOL, 4))
    b2r = np.ascontiguousarray(b2r.reshape(128, NCOL * 4))
    in_maps = []
    for c in range(N_CORES):
        in_maps.append({
            "mem": np.ascontiguousarray(memory[c], np.float32),
            "rp": np.ascontiguousarray(ref_polys[c], np.float32),
            "lv": np.ascontiguousarray(ref_levels[c], np.int32),
            "w1": np.ascontiguousarray(W1, np.float32),
            "b1": np.ascontiguousarray(b1, np.float32),
            "w2": np.ascontiguousarray(W2, np.float32),
            "b2r": b2r,
            "lamr": lamr,
        })
    return in_maps


def kernel(ref_polys, ref_levels, memory, W1, b1, W2, b2, _results_hook=None):
    nc = _get_nc()
    in_maps = make_in_maps(ref_polys, ref_levels, memory, W1, b1, W2, b2)
    res = run_bass_kernel_spmd(nc, in_maps, core_ids=list(range(N_CORES)))
    if _results_hook is not None:
        _results_hook(res)
    outs = [res.results[c]["out"] for c in range(N_CORES)]
    full = np.stack(outs, 0).reshape(B, Q, S * NP, 2).astype(np.float32)
    return full


# revision 13
# speedup vs baseline: 1.0569x; 1.0569x over previous
"""Trainium2 Bass kernel for nn_Decoder_55164559950059 (sparse deformable-attention decoder).

Strategy (data-parallel over batch, one batch element per NeuronCore):
  - Per core: evaluate sampling polynomial on DVE, derive bilinear gather
    indices/weights on DVE, gather 2x2 pixel patches (as two 2-row pair
    fetches) straight from HBM with indirect DMA, combine corners with
    per-partition scalar multiplies, transpose emb via the PE, run the
    2-layer tanh MLP on the PE, and assemble the final reference points.
  - Row ordering: r = s*512 + q (q padded 500->512). Row r lives at
    partition p = q%128... precisely q = qb*128 + p, gather/compute column
    col = s*4 + qb, so tile [128, col] layouts hold everything.

kernel(**inputs) takes the FULL unsharded inputs and returns the full
(8, 500, 16, 2) float32 output.
"""

import sys
from contextlib import ExitStack

import numpy as np

sys.path.insert(0, "/opt/trn_rl_repo")

import concourse.bacc as bacc
import concourse.bass as bass
import concourse.mybir as mybir
import concourse.tile as tile
from concourse.bass_utils import run_bass_kernel_spmd
from concourse.masks import make_identity

from concourse import library_config

f32 = mybir.dt.float32
i32 = mybir.dt.int32
i16 = mybir.dt.int16
Alu = mybir.AluOpType
ActF = mybir.ActivationFunctionType

# ---- problem constants (hardcoded per the harness contract) ----
DEGREE = 3
S = 8                      # sampling points
NP = 2                     # deformable offsets per point
S_SCALE = 0.077
SPATIAL = [(80, 80), (40, 40), (20, 20), (10, 10)]
STARTS = [0, 6400, 8000, 8400]
B, Q, C = 8, 500, 256
HWR = 8500                 # total memory rows per batch
QP = 512                   # padded queries
NQB = QP // 128            # 4 query blocks
NCOL = S * NQB             # 32 chunk columns, col = s*NQB + qb
NGRP = NCOL // 4           # 8 groups of 4 columns (512 rows each)
TWO23 = 12582912.0         # 1.5*2**23 fp32 round-to-int bias (spacing 1.0 at the sum)

N_CORES = 8


def _lambdas_np():
    lam = np.linspace(0.0, 1.0, S, dtype=np.float32)
    return np.stack([lam ** i for i in range(DEGREE, -1, -1)], 1)  # (S, 4)


def _level_quantity(vals):
    """vals[l] for l in 0..3 -> list of (level, value) floats."""
    return [float(v) for v in vals]


def build_nc():
    nc = bacc.Bacc("TRN2", target_bir_lowering=False, debug=False)

    mem = nc.dram_tensor("mem", [HWR, C], f32, kind="ExternalInput")
    rp = nc.dram_tensor("rp", [Q, 2 * (DEGREE + 1)], f32, kind="ExternalInput")
    lv = nc.dram_tensor("lv", [Q], i32, kind="ExternalInput")
    w1 = nc.dram_tensor("w1", [C, C], f32, kind="ExternalInput")
    b1 = nc.dram_tensor("b1", [C], f32, kind="ExternalInput")
    w2 = nc.dram_tensor("w2", [C, 2 * NP], f32, kind="ExternalInput")
    b2r = nc.dram_tensor("b2r", [128, NCOL * 4], f32, kind="ExternalInput")
    lamr = nc.dram_tensor("lamr", [128, 256], f32, kind="ExternalInput")
    out = nc.dram_tensor("out", [Q, S, NP, 2], f32, kind="ExternalOutput")

    with tile.TileContext(nc) as tc:
        with ExitStack() as ctx:
            _body(ctx, tc, mem, rp, lv, w1, b1, w2, b2r, lamr, out)

    nc.compile()
    return nc


def _body(ctx, tc, mem, rp, lv, w1, b1, w2, b2r, lamr, out):
    nc = tc.nc
    v = nc.vector
    sc = nc.scalar
    gp = nc.gpsimd

    cpool = ctx.enter_context(tc.tile_pool(name="consts", bufs=1))
    spool = ctx.enter_context(tc.tile_pool(name="scratch", bufs=1))
    gpool = ctx.enter_context(tc.tile_pool(name="gather", bufs=4))
    epool = ctx.enter_context(tc.tile_pool(name="emb", bufs=3))
    pepool = ctx.enter_context(tc.tile_pool(name="pe_t", bufs=2, space="PSUM"))
    esb = ctx.enter_context(tc.tile_pool(name="embT", bufs=2))
    hpsum = ctx.enter_context(tc.tile_pool(name="hpsum", bufs=2, space="PSUM"))
    hsb = ctx.enter_context(tc.tile_pool(name="hT", bufs=2))
    opsum = ctx.enter_context(tc.tile_pool(name="opsum", bufs=1, space="PSUM"))
    fpool = ctx.enter_context(tc.tile_pool(name="final", bufs=1))

    # ---------------- constants & weights ----------------
    gp.load_library(library_config.mlp)

    ident = cpool.tile([128, 128], f32, tag="ident")
    make_identity(nc, ident[:, :])

    w1_sb = [cpool.tile([128, C], f32, tag=f"w1_{kh}", name=f"w1_sb{kh}") for kh in range(2)]
    for kh in range(2):
        nc.sync.dma_start(w1_sb[kh][:, :], w1[kh * 128:(kh + 1) * 128, :])
    w2_sb = [cpool.tile([128, 2 * NP], f32, tag=f"w2_{kh}", name=f"w2_sb{kh}") for kh in range(2)]
    for kh in range(2):
        nc.sync.dma_start(w2_sb[kh][:, :], w2[kh * 128:(kh + 1) * 128, :])
    b1_sb = cpool.tile([128, 2], f32, tag="b1")
    nc.sync.dma_start(b1_sb[:, :], b1.rearrange("(h p) -> p h", p=128))
    b2r_sb = cpool.tile([128, NCOL * 4], f32, tag="b2r")
    nc.sync.dma_start(b2r_sb[:, :], b2r[:, :])
    lam_sb = cpool.tile([128, 256], f32, tag="lamr")
    nc.sync.dma_start(lam_sb[:, :], lamr[:, :])

    # ---------------- small input loads (ragged 500 -> [128, 4]) ----------------
    rp_t = spool.tile([128, NQB, 8], f32, tag="rp_t")
    v.memset(rp_t[:, :, :], 0.0)
    nc.sync.dma_start(rp_t[:, 0:3, :], rp[0:384, :].rearrange("(a p) m -> p a m", p=128))
    nc.sync.dma_start(rp_t[0:116, 3, :], rp[384:500, :])

    lv_t = spool.tile([128, NQB], i32, tag="lv_t")
    v.memset(lv_t[:, :], 0)
    nc.sync.dma_start(lv_t[:, 0:3], lv[0:384].rearrange("(a p) -> p a", p=128))
    nc.sync.dma_start(lv_t[0:116, 3], lv[384:500])

    # ---------------- polynomial eval: e[p, s, qb, c] ----------------
    rp_rep = spool.tile([128, S, NQB, 2, DEGREE + 1], f32, tag="rp_rep")
    for s in range(S):
        v.tensor_copy(rp_rep[:, s, :, :, :], rp_t[:, :, :].rearrange("p a (c d) -> p a c d", c=2))
    prod = spool.tile([128, S, NQB, 2, DEGREE + 1], f32, tag="prod")
    v.tensor_tensor(
        out=prod[:, :, :, :, :],
        in0=rp_rep[:, :, :, :, :],
        in1=lam_sb[:, :].rearrange("p (s a c d) -> p s a c d", s=S, a=NQB, c=2),
        op=Alu.mult,
    )
    e_t = spool.tile([128, S, NQB, 2], f32, tag="e_t")
    v.tensor_reduce(e_t[:, :, :, :], prod[:, :, :, :, :], axis=mybir.AxisListType.X, op=Alu.add)

    # sp = 2*e - 1 (sampling points in [-1, 1], layout [p, s, qb, c])
    sp_t = spool.tile([128, S, NQB, 2], f32, tag="sp_t")
    v.tensor_scalar(out=sp_t[:, :, :, :], in0=e_t[:, :, :, :],
                    scalar1=2.0, scalar2=-1.0, op0=Alu.mult, op1=Alu.add)

    # ---------------- per-query level constants at [128, NCOL] ----------------
    lv_f = spool.tile([128, NQB], f32, tag="lv_f")
    v.tensor_copy(lv_f[:, :], lv_t[:, :])
    lv_rep = spool.tile([128, S, NQB], f32, tag="lv_rep")
    for s in range(S):
        v.tensor_copy(lv_rep[:, s, :], lv_f[:, :])

    def level_tile(tag, vals):
        acc = spool.tile([128, S, NQB], f32, tag=tag)
        tmp = spool.tile([128, S, NQB], f32, tag=tag + "_tmp")
        for l in range(4):
            dst = acc if l == 0 else tmp
            v.tensor_scalar(out=dst[:, :, :], in0=lv_rep[:, :, :],
                            scalar1=float(l), scalar2=float(vals[l]),
                            op0=Alu.is_equal, op1=Alu.mult)
            if l > 0:
                v.tensor_tensor(out=acc[:, :, :], in0=acc[:, :, :], in1=tmp[:, :, :], op=Alu.add)
        return acc

    Wq = level_tile("Wq", [wd for (_, wd) in SPATIAL])
    Hq = level_tile("Hq", [h for (h, _) in SPATIAL])
    Wm2 = level_tile("Wm2", [wd - 2 for (_, wd) in SPATIAL])
    Hm2 = level_tile("Hm2", [h - 2 for (h, _) in SPATIAL])
    STq = level_tile("STq", STARTS)

    # ---------------- bilinear index / weight math ----------------
    def axis_math(e_comp, SZ, SZm2, tag):
        """e_comp: [128, S, NQB] view of e (x or y). Returns (lo, wA, wB)."""
        g = spool.tile([128, S, NQB], f32, tag=tag + "_g")
        v.tensor_tensor(out=g[:, :, :], in0=e_comp, in1=SZ[:, :, :], op=Alu.mult)
        v.tensor_scalar_add(g[:, :, :], g[:, :, :], -0.5)
        # exact floor via 2^23 round + correction
        r = spool.tile([128, S, NQB], f32, tag=tag + "_r")
        v.tensor_scalar(out=r[:, :, :], in0=g[:, :, :], scalar1=TWO23, scalar2=TWO23,
                        op0=Alu.add, op1=Alu.subtract)
        cmp = spool.tile([128, S, NQB], f32, tag=tag + "_c")
        v.tensor_tensor(out=cmp[:, :, :], in0=r[:, :, :], in1=g[:, :, :], op=Alu.is_gt)
        x0 = spool.tile([128, S, NQB], f32, tag=tag + "_x0")
        v.tensor_tensor(out=x0[:, :, :], in0=r[:, :, :], in1=cmp[:, :, :], op=Alu.subtract)
        fx = spool.tile([128, S, NQB], f32, tag=tag + "_f")
        v.tensor_tensor(out=fx[:, :, :], in0=g[:, :, :], in1=x0[:, :, :], op=Alu.subtract)
        lo = spool.tile([128, S, NQB], f32, tag=tag + "_lo")
        v.tensor_scalar_max(lo[:, :, :], x0[:, :, :], 0.0)
        v.tensor_tensor(out=lo[:, :, :], in0=lo[:, :, :], in1=SZm2[:, :, :], op=Alu.min)
        x0p = spool.tile([128, S, NQB], f32, tag=tag + "_x0p")
        v.tensor_scalar_add(x0p[:, :, :], x0[:, :, :], 1.0)
        lop = spool.tile([128, S, NQB], f32, tag=tag + "_lop")
        v.tensor_scalar_add(lop[:, :, :], lo[:, :, :], 1.0)

        def eqw(a, b, tg):
            t = spool.tile([128, S, NQB], f32, tag=tg)
            v.tensor_tensor(out=t[:, :, :], in0=a[:, :, :], in1=b[:, :, :], op=Alu.is_equal)
            return t

        e1 = eqw(lo, x0, tag + "_e1")
        e2 = eqw(lo, x0p, tag + "_e2")
        e3 = eqw(lop, x0, tag + "_e3")
        e4 = eqw(lop, x0p, tag + "_e4")

        def blend(ea, eb, tg):
            # w = ea + fx*(eb - ea)
            d = spool.tile([128, S, NQB], f32, tag=tg + "_d")
            v.tensor_tensor(out=d[:, :, :], in0=eb[:, :, :], in1=ea[:, :, :], op=Alu.subtract)
            v.tensor_tensor(out=d[:, :, :], in0=fx[:, :, :], in1=d[:, :, :], op=Alu.mult)
            v.tensor_tensor(out=d[:, :, :], in0=ea[:, :, :], in1=d[:, :, :], op=Alu.add)
            return d

        wA = blend(e1, e2, tag + "_wA")
        wB = blend(e3, e4, tag + "_wB")
        return lo, wA, wB

    xlo, wxA, wxB = axis_math(e_t[:, :, :, 0], Wq, Wm2, "x")
    ylo, wyA, wyB = axis_math(e_t[:, :, :, 1], Hq, Hm2, "y")

    def outer(wy, wx, tg):
        t = spool.tile([128, S, NQB], f32, tag=tg)
        v.tensor_tensor(out=t[:, :, :], in0=wy[:, :, :], in1=wx[:, :, :], op=Alu.mult)
        return t

    wAA = outer(wyA, wxA, "wAA")   # row yl,   col xl / xl+1
    wAB = outer(wyA, wxB, "wAB")
    wBA = outer(wyB, wxA, "wBA")
    wBB = outer(wyB, wxB, "wBB")

    rowA = spool.tile([128, S, NQB], f32, tag="rowA")
    v.tensor_tensor(out=rowA[:, :, :], in0=ylo[:, :, :], in1=Wq[:, :, :], op=Alu.mult)
    v.tensor_tensor(out=rowA[:, :, :], in0=rowA[:, :, :], in1=STq[:, :, :], op=Alu.add)
    v.tensor_tensor(out=rowA[:, :, :], in0=rowA[:, :, :], in1=xlo[:, :, :], op=Alu.add)
    rowB = spool.tile([128, S, NQB], f32, tag="rowB")
    v.tensor_tensor(out=rowB[:, :, :], in0=rowA[:, :, :], in1=Wq[:, :, :], op=Alu.add)

    # int16 indices, free layout (s, qb, ab) -> global gather position
    # I = p + 128*(8*s + 2*qb + ab); dma_gather wants wrapped[I%16, I//16]
    # replicated across the 8 Q7 core groups.
    idx16 = spool.tile([128, S, NQB, 2], i16, tag="idx16")
    v.tensor_copy(idx16[:, :, :, 0], rowA[:, :, :])
    v.tensor_copy(idx16[:, :, :, 1], rowB[:, :, :])
    idx16_f = idx16[:, :, :, :].rearrange("p s a j -> p (s a j)")  # [128, 64]
    w16 = spool.tile([16, 64, 8], i16, tag="w16")  # [pl, cglobal, pg]
    for pg in range(8):
        nc.sync.dma_start(w16[:, :, pg], idx16_f[16 * pg:16 * (pg + 1), :])
    idxs_t = spool.tile([128, 512], i16, tag="idxs_t")
    w16_f = w16[:, :, :].rearrange("p c g -> p (c g)")  # [16, 512]
    for k in range(8):
        nc.sync.dma_start(idxs_t[16 * k:16 * (k + 1), :], w16_f)

    mem_pairs = bass.AP(tensor=mem[:, :].tensor, offset=0, ap=[[C, HWR - 1], [1, 2 * C]])

    # corner weights packed [p, col, corner, 1] for per-chunk diag builds
    wplan = spool.tile([128, NCOL, 4, 1], f32, tag="wplan")
    for k, wt in enumerate((wAA, wAB, wBA, wBB)):
        v.tensor_copy(wplan[:, :, k, 0], wt[:, :, :].rearrange("p s a -> p (s a)"))

    # 4 identity matrices side by side (rhs for the 4 corner transpose-MMs)
    i4 = cpool.tile([128, 4, 128], f32, tag="i4")
    for k in range(4):
        v.tensor_copy(i4[:, k, :], ident[:, :])

    # ---------------- main loop: gather -> combine -> transpose -> MLP ----------------
    psum_off = opsum.tile([128, NCOL * 4], f32, tag="psum_off")

    for gg in range(8):
        g_t = gpool.tile([128, 8, 2 * C], f32, tag="g_t")
        gp.dma_gather(
            out_ap=g_t[:, :, :],
            in_ap=mem_pairs,
            idxs_ap=idxs_t[:, 64 * gg:64 * (gg + 1)],
            num_idxs=1024,
            num_idxs_reg=1024,
            elem_size=2 * C,
            elem_step=C,
        )
        for g2 in range(1):
            g = gg
            embT_sb = [esb.tile([128, 512], f32, tag=f"embT_{kh}", name=f"embT_sb{kh}_{g}") for kh in range(2)]
            for j in range(4):
                col = 4 * g + j
                pl = 2 * j
                # diag(w_k) for the 4 corners of this chunk, one fused build
                diag4 = epool.tile([128, 4, 128], f32, tag="diag4")
                v.tensor_tensor(out=diag4[:, :, :], in0=i4[:, :, :],
                                in1=wplan[:, col, :, :].to_broadcast([128, 4, 128]),
                                op=Alu.mult)
                # fused bilinear combine + transpose on the PE:
                # embT[:, kh*128:] += sum_k g_corner_k.T @ diag(w_k)
                pet = pepool.tile([128, 2, 128], f32, tag="pet")
                for kh in range(2):
                    for k, (dpl, pix) in enumerate(((pl, 0), (pl, 1), (pl + 1, 0), (pl + 1, 1))):
                        nc.tensor.matmul(
                            pet[:, kh, :],
                            lhsT=g_t[:, dpl, pix * C + kh * 128:pix * C + kh * 128 + 128],
                            rhs=diag4[:, k, :],
                            start=(k == 0),
                            stop=(k == 3),
                        )
                v.tensor_copy(embT_sb[0][:, j * 128:(j + 1) * 128], pet[:, 0, :])
                sc.copy(embT_sb[1][:, j * 128:(j + 1) * 128], pet[:, 1, :])

            # MLP layer 1: h = tanh(emb @ W1 + b1), computed transposed
            hT_sb = [hsb.tile([128, 512], f32, tag=f"hT_{mh}", name=f"hT_sb{mh}_{g}") for mh in range(2)]
            for mh in range(2):
                ph = hpsum.tile([128, 512], f32, tag=f"ph_{mh}")
                for kh in range(2):
                    nc.tensor.matmul(
                        ph[:, :],
                        lhsT=w1_sb[kh][:, mh * 128:(mh + 1) * 128],
                        rhs=embT_sb[kh][:, :],
                        start=(kh == 0),
                        stop=(kh == 1),
                    )
                sc.activation(hT_sb[mh][:, :], ph[:, :], ActF.Tanh, bias=b1_sb[:, mh:mh + 1], scale=1.0)

            # MLP layer 2 (swapped operands): off[rows, 4] directly
            for j in range(4):
                col = 4 * g + j
                for kh in range(2):
                    nc.tensor.matmul(
                        psum_off[:, col * 4:(col + 1) * 4],
                        lhsT=hT_sb[kh][:, j * 128:(j + 1) * 128],
                        rhs=w2_sb[kh][:, :],
                        start=(kh == 0),
                        stop=(kh == 1),
                    )

    # ---------------- final: ref = S_SCALE*tanh(off + b2) + sp ----------------
    spts4 = fpool.tile([128, NCOL, NP, 2], f32, tag="spts4")
    sp_cols = sp_t[:, :, :, :].rearrange("p s a c -> p (s a) c")
    for np_i in range(NP):
        v.tensor_copy(spts4[:, :, np_i, :], sp_cols)

    so1 = fpool.tile([128, NCOL * 4], f32, tag="so1")
    v.tensor_tensor(out=so1[:, :], in0=psum_off[:, :], in1=b2r_sb[:, :], op=Alu.add)
    so2 = fpool.tile([128, NCOL * 4], f32, tag="so2")
    sc.activation(so2[:, :], so1[:, :], ActF.Tanh)
    ref_out = fpool.tile([128, NCOL, NP, 2], f32, tag="ref_out")
    v.tensor_scalar_mul(ref_out[:, :, :, :],
                        so2[:, :].rearrange("p (k n c) -> p k n c", n=NP, c=2),
                        S_SCALE)
    v.tensor_tensor(out=ref_out[:, :, :, :], in0=ref_out[:, :, :, :],
                    in1=spts4[:, :, :, :], op=Alu.add)

    # ---------------- output DMA (ragged 512 -> 500) ----------------
    ro = ref_out[:, :, :, :].rearrange("p (s a) n c -> p s a (n c)", s=S)
    for qb in range(NQB):
        pmax = 128 if qb < 3 else 116
        nc.sync.dma_start(
            out[qb * 128:qb * 128 + pmax, :, :, :].rearrange("p s n c -> p s (n c)"),
            ro[0:pmax, :, qb, :],
        )


# ---------------- host wrapper ----------------
_NC_CACHE = {}


def _get_nc():
    if "nc" not in _NC_CACHE:
        _NC_CACHE["nc"] = build_nc()
    return _NC_CACHE["nc"]


def make_in_maps(ref_polys, ref_levels, memory, W1, b1, W2, b2):
    lam = _lambdas_np()  # (S, 4)
    lamr = np.broadcast_to(lam[None, :, None, None, :], (128, S, NQB, 2, DEGREE + 1))
    lamr = np.ascontiguousarray(lamr.reshape(128, 256).astype(np.float32))
    b2r = np.broadcast_to(np.asarray(b2, np.float32)[None, None, :], (128, NCOL, 4))
    b2r = np.ascontiguousarray(b2r.reshape(128, NCOL * 4))
    in_maps = []
    for c in range(N_CORES):
        in_maps.append({
            "mem": np.ascontiguousarray(memory[c], np.float32),
            "rp": np.ascontiguousarray(ref_polys[c], np.float32),
            "lv": np.ascontiguousarray(ref_levels[c], np.int32),
            "w1": np.ascontiguousarray(W1, np.float32),
            "b1": np.ascontiguousarray(b1, np.float32),
            "w2": np.ascontiguousarray(W2, np.float32),
            "b2r": b2r,
            "lamr": lamr,
        })
    return in_maps


def kernel(ref_polys, ref_levels, memory, W1, b1, W2, b2, _results_hook=None):
    nc = _get_nc()
    in_maps = make_in_maps(ref_polys, ref_levels, memory, W1, b1, W2, b2)
    res = run_bass_kernel_spmd(nc, in_maps, core_ids=list(range(N_CORES)))
    if _results_hook is not None:
        _results_hook(res)
    outs = [res.results[c]["out"] for c in range(N_CORES)]
    full = np.stack(outs, 0).reshape(B, Q, S * NP, 2).astype(np.float32)
    return full
